# revision 6
# baseline (speedup 1.0000x reference)
"""Trainium2 Bass kernel for nn_CNNNer (sparse band biaffine NER scorer).

Math collapse used here (everything after the GELU stage is linear):
  head = gelu(state@Wh+bh) ++ [1]          (features i = 0..200, i=200 is the 1)
  tail = gelu(state@Wt+bt) ++ [1]
  band[n,r,k] = head[n]^T U''_k tail[m],  m = n+r-64
      with U''_k = U_k + e_200 Wtp[k,:] + Whp[k,:]^T e_200^T
      (folds the h2/t2 additive terms of scores2 through the ones feature)
  scores'[n,r,t] = sum_k Wd[k,t] band_masked[n,r,k]
      masking zeroes whole head/tail feature columns (query/key validity),
      which commutes with the k-contraction, so
  scores'[n,r,t] = head_masked[n]^T UW_t tail_masked[m],
      UW_t = sum_k Wd[k,t] U''_k            (precomputed on host, [9,201,201])
  scores = scores' + bd  (host), masked-out entries = bd exactly.

Device work per core (8 cores; core = (batch b, query quarter) of 256 queries):
  1. headT/tailT = gelu MLPs computed transposed ([feature, position]).
  2. step A: UhT_t[j, x] = sum_i UW[t,i,j] headT[i,x]        (9 tags)
  3. step B: S_t[x, m]  = sum_j UhT_t[j, x] tailT[j, m]      (full 128x256
     score windows per query-chunk; band diag extracted on host)
"""

import os

import numpy as np

B, N, HID = 2, 1024, 768
BSZ = 200
W = 64
TAGS = 9
F = BSZ + 1  # 201 features incl the ones column
NQ = 256  # queries per core
NW = NQ + 2 * W  # 384 window positions per core
R = 2 * W + 1  # 129 band offsets
NCORES = 8
I2 = F - 128  # 73: second feature tile rows (i = 128..200)
F2 = BSZ - 128  # 72: second MLP output tile cols

_cache: dict = {}


def _build_nc():
    import concourse.mybir as mybir
    import concourse.tile as tile
    from concourse import bacc

    dt = mybir.dt
    f32 = dt.float32
    mm_dt_name = os.environ.get("BASSK_MM_DT", "f32")
    mm_dt = {"f32r": dt.float32r, "f32": dt.float32}[mm_dt_name]

    def mm(ap):
        return ap.bitcast(mm_dt) if mm_dt != f32 else ap

    nc = bacc.Bacc(
        "TRN2", target_bir_lowering=False, debug=False, enable_asserts=False
    )
    xT = nc.dram_tensor("xT", [HID, NW], f32, kind="ExternalInput").ap()
    wh = nc.dram_tensor("wh", [HID, BSZ], f32, kind="ExternalInput").ap()
    wt = nc.dram_tensor("wt", [HID, BSZ], f32, kind="ExternalInput").ap()
    bh = nc.dram_tensor("bh", [BSZ], f32, kind="ExternalInput").ap()
    bt = nc.dram_tensor("bt", [BSZ], f32, kind="ExternalInput").ap()
    uw = nc.dram_tensor("uw", [TAGS, F, F], f32, kind="ExternalInput").ap()
    msk = nc.dram_tensor("msk", [1, NW], f32, kind="ExternalInput").ap()
    sout = nc.dram_tensor("sout", [TAGS, NQ, NQ], f32, kind="ExternalOutput").ap()

    gelu = {
        "gelu": mybir.ActivationFunctionType.Gelu,
        "identity": mybir.ActivationFunctionType.Identity,
    }[os.environ.get("BASSK_ACT", "gelu")]

    with tile.TileContext(nc) as tc:
        with (
            tc.tile_pool(name="sb", bufs=1) as sb,
            tc.tile_pool(name="ps_mlp", bufs=2, space="PSUM") as ps_mlp,
            tc.tile_pool(name="ps_a", bufs=2, space="PSUM") as ps_a,
            tc.tile_pool(name="ps_s", bufs=4, space="PSUM") as ps_s,
        ):
            # ---- loads ----
            x_sb = sb.tile([128, 6, NW], f32)
            nc.sync.dma_start(out=x_sb, in_=xT.rearrange("(ht p) c -> p ht c", p=128))
            wh_sb = sb.tile([128, 6, BSZ], f32)
            nc.sync.dma_start(out=wh_sb, in_=wh.rearrange("(ht p) m -> p ht m", p=128))
            wt_sb = sb.tile([128, 6, BSZ], f32)
            nc.sync.dma_start(out=wt_sb, in_=wt.rearrange("(ht p) m -> p ht m", p=128))
            uw1 = sb.tile([128, TAGS, F], f32)
            nc.sync.dma_start(out=uw1, in_=uw[:, 0:128, :].transpose([1, 0, 2]))
            uw2 = sb.tile([I2, TAGS, F], f32)
            nc.sync.dma_start(out=uw2, in_=uw[:, 128:F, :].transpose([1, 0, 2]))
            m_sb = sb.tile([128, NW], f32)
            nc.sync.dma_start(out=m_sb, in_=msk.partition_broadcast(128))
            bh1 = sb.tile([128, 1], f32)
            nc.sync.dma_start(out=bh1, in_=bh[0:128].unsqueeze(1))
            bh2 = sb.tile([F2, 1], f32)
            nc.sync.dma_start(out=bh2, in_=bh[128:BSZ].unsqueeze(1))
            bt1 = sb.tile([128, 1], f32)
            nc.sync.dma_start(out=bt1, in_=bt[0:128].unsqueeze(1))
            bt2 = sb.tile([F2, 1], f32)
            nc.sync.dma_start(out=bt2, in_=bt[128:BSZ].unsqueeze(1))

            headT1 = sb.tile([128, NQ], f32)
            headT2 = sb.tile([I2, NQ], f32)
            tailT1 = sb.tile([128, NW], f32)
            tailT2 = sb.tile([I2, NW], f32)
            uh1 = sb.tile([128, TAGS, NQ], f32)
            uh2 = sb.tile([I2, TAGS, NQ], f32)
            s_sb0 = sb.tile([128, TAGS, NQ], f32)
            s_sb1 = sb.tile([128, TAGS, NQ], f32)

            # ---- MLPs: o = gelu(W^T x + b), computed transposed ----
            for w_sb, b1, b2, o1, o2, c0, ncols in (
                (wh_sb, bh1, bh2, headT1, headT2, W, NQ),
                (wt_sb, bt1, bt2, tailT1, tailT2, 0, NW),
            ):
                for fw, f0, o, bias in ((128, 0, o1, b1), (F2, 128, o2, b2)):
                    pm = ps_mlp.tile([fw, ncols], f32, tag="pm")
                    for ht in range(6):
                        nc.tensor.matmul(
                            pm,
                            mm(w_sb[:, ht, f0 : f0 + fw]),
                            mm(x_sb[:, ht, c0 : c0 + ncols]),
                            start=(ht == 0),
                            stop=(ht == 5),
                        )
                    nc.scalar.activation(out=o[0:fw, :], in_=pm, func=gelu, bias=bias)
                # mask all columns; ones feature row (i == 200) is the mask
                # row itself, DMA'd in (engines can't address partition 72)
                nc.vector.tensor_mul(o1, o1, m_sb[0:128, c0 : c0 + ncols])
                nc.vector.tensor_mul(
                    o2[0:F2, :], o2[0:F2, :], m_sb[0:F2, c0 : c0 + ncols]
                )
                nc.sync.dma_start(
                    out=o2[F2 : F2 + 1, :], in_=msk[:, c0 : c0 + ncols]
                )

            # ---- step A: UhT_t[j, x] = sum_i UW[t,i,j] headT[i,x] ----
            for t in range(TAGS):
                for jw, j0, uh in ((128, 0, uh1), (I2, 128, uh2)):
                    pa = ps_a.tile([jw, NQ], f32, tag="pa")
                    for it, (u_sb, h_sb) in enumerate(
                        ((uw1, headT1), (uw2, headT2))
                    ):
                        nc.tensor.matmul(
                            pa,
                            mm(u_sb[:, t, j0 : j0 + jw]),
                            mm(h_sb),
                            start=(it == 0),
                            stop=(it == 1),
                        )
                    nc.any.tensor_copy(uh[:, t, :], pa)

            # ---- step B: S_t[x, m] = sum_j UhT_t[j, x] tailT[j, m] ----
            for qc in range(2):
                s_sb = (s_sb0, s_sb1)[qc]
                for t in range(TAGS):
                    pS = ps_s.tile([128, NQ], f32, tag="ps")
                    for jt, (uh, tl) in enumerate(((uh1, tailT1), (uh2, tailT2))):
                        nc.tensor.matmul(
                            pS,
                            mm(uh[:, t, qc * 128 : qc * 128 + 128]),
                            mm(tl[:, qc * 128 : qc * 128 + NQ]),
                            start=(jt == 0),
                            stop=(jt == 1),
                        )
                    nc.any.tensor_copy(s_sb[:, t, :], pS)
                nc.sync.dma_start(
                    out=sout[:, qc * 128 : (qc + 1) * 128, :].transpose([1, 0, 2]),
                    in_=s_sb,
                )

    nc.compile()
    return nc


def _get_nc():
    if "nc" not in _cache:
        _cache["nc"] = _build_nc()
    return _cache["nc"]


def _install_ntff_hook():
    """Profiling-only (BASSK_TRACE=1): provide antenv.axon_hooks if the
    image lacks it, wired to the libaxon NTFF capture via ctypes."""
    import sys
    import types

    try:
        from antenv.axon_hooks import get_axon_ntff_profile_hook  # noqa: F401

        return
    except ImportError:
        pass
    from trn_agent_boot.trn_boot import _ntff_profile_via_ctypes

    hook = _ntff_profile_via_ctypes("/opt/axon/libaxon_pjrt.so")
    mod = types.ModuleType("antenv.axon_hooks")
    mod._hook = hook
    mod.get_axon_ntff_profile_hook = lambda: mod._hook
    mod.set_axon_ntff_profile_hook = lambda h: setattr(mod, "_hook", h)
    sys.modules["antenv.axon_hooks"] = mod


def _host_prep(state, lengths, Wh, bh, Wt, bt, U, Wcat, Wd):
    """Fold U/Wcat/Wd into UW[9,201,201] and build per-core inputs."""
    Whp = Wcat[:, :F]  # [K, 201]
    Wtp = Wcat[:, F:]  # [K, 201]
    U2 = U.astype(np.float64).copy()
    U2[:, F - 1, :] += Wtp  # head ones-row picks up the tail term
    U2[:, :, F - 1] += Whp  # tail ones-col picks up the head term
    UW = np.einsum("kt,kij->tij", Wd.astype(np.float64), U2).astype(np.float32)
    UW = np.ascontiguousarray(UW)

    in_maps = []
    for b in range(B):
        for qi in range(N // NQ):
            q0 = qi * NQ
            lo = q0 - W
            xw = np.zeros((NW, HID), np.float32)
            s, e = max(lo, 0), min(q0 + NQ + W, N)
            xw[s - lo : e - lo] = state[b, s:e]
            pos = lo + np.arange(NW)
            mrow = ((pos >= 0) & (pos < N) & (pos < lengths[b])).astype(np.float32)
            in_maps.append(
                {
                    "xT": np.ascontiguousarray(xw.T),
                    "wh": Wh,
                    "wt": Wt,
                    "bh": bh,
                    "bt": bt,
                    "uw": UW,
                    "msk": np.ascontiguousarray(mrow[None, :]),
                }
            )
    return in_maps


def _assemble(outs, bd):
    """outs: NCORES arrays [TAGS, NQ, NQ] -> scores [B, N, R, TAGS]."""
    scores = np.empty((B, N, R, TAGS), np.float32)
    xg = np.arange(NQ)
    mi = (xg % 128)[:, None] + np.arange(R)[None, :]  # [NQ, R] col gather idx
    for c, S in enumerate(outs):
        b, qi = divmod(c, N // NQ)
        g = np.take_along_axis(S, mi[None, :, :], axis=2)  # [TAGS, NQ, R]
        scores[b, qi * NQ : (qi + 1) * NQ] = g.transpose(1, 2, 0)
    scores += bd.astype(np.float32)[None, None, None, :]
    return np.where(np.isfinite(scores), scores, 0.0).astype(np.float32)


def kernel(**inputs):
    state = np.asarray(inputs["state"], np.float32)
    lengths = np.asarray(inputs["lengths"]).astype(np.int64)
    Wh = np.ascontiguousarray(np.asarray(inputs["Wh"], np.float32))
    bh = np.asarray(inputs["bh"], np.float32)
    Wt = np.ascontiguousarray(np.asarray(inputs["Wt"], np.float32))
    bt = np.asarray(inputs["bt"], np.float32)
    U = np.asarray(inputs["U"], np.float32)
    Wcat = np.asarray(inputs["Wcat"], np.float32)
    Wd = np.asarray(inputs["Wd"], np.float32)
    bd = np.asarray(inputs["bd"], np.float32)

    in_maps = _host_prep(state, lengths, Wh, bh, Wt, bt, U, Wcat, Wd)
    nc = _get_nc()

    if os.environ.get("BASSK_SIM"):
        from concourse.bass_interp import CoreSim

        outs = []
        for im in in_maps:
            sim = CoreSim(nc, trace=False)
            for k, v in im.items():
                sim.tensor(k)[:] = v
            sim.simulate()
            outs.append(sim.tensor("sout").copy())
    else:
        trace = bool(os.environ.get("BASSK_TRACE"))
        if trace:
            _install_ntff_hook()
        from concourse.bass_utils import run_bass_kernel_spmd

        res = run_bass_kernel_spmd(
            nc,
            in_maps,
            core_ids=list(range(NCORES)),
            trace=trace,
        )
        _cache["last_result"] = res
        outs = [r["sout"] for r in res.results]

    return _assemble(outs, bd)


# revision 11
# speedup vs baseline: 1.3099x; 1.3099x over previous
"""Trainium2 Bass kernel for nn_CNNNer (sparse band biaffine NER scorer).

Math collapse used here (everything after the GELU stage is linear):
  head = gelu(state@Wh+bh) ++ [1]          (features i = 0..200, i=200 is the 1)
  tail = gelu(state@Wt+bt) ++ [1]
  band[n,r,k] = head[n]^T U''_k tail[m],  m = n+r-64
      with U''_k = U_k + e_200 Wtp[k,:] + Whp[k,:]^T e_200^T
      (folds the h2/t2 additive terms of scores2 through the ones feature)
  scores'[n,r,t] = sum_k Wd[k,t] band_masked[n,r,k]
      masking zeroes whole head/tail feature columns (query/key validity),
      which commutes with the k-contraction, so
  scores'[n,r,t] = head_masked[n]^T UW_t tail_masked[m],
      UW_t = sum_k Wd[k,t] U''_k            (precomputed on host, [9,201,201])
  scores = scores' + bd  (host), masked-out entries = bd exactly.

Device work per core (8 cores; core = (batch b, query quarter) of 256 queries):
  1. headT/tailT = gelu MLPs computed transposed ([feature, position]).
  2. step A: UhT_t[j, x] = sum_i UW[t,i,j] headT[i,x]        (9 tags)
  3. step B: S_t[x, m]  = sum_j UhT_t[j, x] tailT[j, m]      (full 128x256
     score windows per query-chunk; band diag extracted on host)
"""

import os

import numpy as np

B, N, HID = 2, 1024, 768
BSZ = 200
W = 64
TAGS = 9
F = BSZ + 1  # 201 features incl the ones column
NQ = 256  # queries per core
NW = NQ + 2 * W  # 384 window positions per core
R = 2 * W + 1  # 129 band offsets
NCORES = 8
I2 = F - 128  # 73: second feature tile rows (i = 128..200)
F2 = BSZ - 128  # 72: second MLP output tile cols

_cache: dict = {}


def io_dt_name():
    return os.environ.get("BASSK_IO_DT", "f32r")


def _build_nc():
    import concourse.mybir as mybir
    import concourse.tile as tile
    from concourse import bacc

    dt = mybir.dt
    f32 = dt.float32
    io = {"f32": f32, "f32r": dt.float32r, "bf16": dt.bfloat16}[io_dt_name()]

    nc = bacc.Bacc(
        "TRN2", target_bir_lowering=False, debug=False, enable_asserts=False
    )
    xT = nc.dram_tensor("xT", [HID, NW], io, kind="ExternalInput").ap()
    wh = nc.dram_tensor("wh", [HID, BSZ], io, kind="ExternalInput").ap()
    wt = nc.dram_tensor("wt", [HID, BSZ], io, kind="ExternalInput").ap()
    bh = nc.dram_tensor("bh", [BSZ], f32, kind="ExternalInput").ap()
    bt = nc.dram_tensor("bt", [BSZ], f32, kind="ExternalInput").ap()
    uw = nc.dram_tensor("uw", [TAGS, F, F], io, kind="ExternalInput").ap()
    msk = nc.dram_tensor("msk", [1, NW], io, kind="ExternalInput").ap()
    sout = nc.dram_tensor("sout", [TAGS, NQ, NQ], f32, kind="ExternalOutput").ap()

    gelu = {
        "gelu": mybir.ActivationFunctionType.Gelu,
        "identity": mybir.ActivationFunctionType.Identity,
    }[os.environ.get("BASSK_ACT", "gelu")]

    with tile.TileContext(nc) as tc:
        with (
            tc.tile_pool(name="sb", bufs=1) as sb,
            tc.tile_pool(name="ps_mlp", bufs=2, space="PSUM") as ps_mlp,
            tc.tile_pool(name="ps_a", bufs=2, space="PSUM") as ps_a,
            tc.tile_pool(name="ps_s", bufs=4, space="PSUM") as ps_s,
        ):
            # ---- loads ----
            x_sb = sb.tile([128, 6, NW], io)
            nc.sync.dma_start(out=x_sb, in_=xT.rearrange("(ht p) c -> p ht c", p=128))
            wh_sb = sb.tile([128, 6, BSZ], io)
            nc.sync.dma_start(out=wh_sb, in_=wh.rearrange("(ht p) m -> p ht m", p=128))
            wt_sb = sb.tile([128, 6, BSZ], io)
            nc.sync.dma_start(out=wt_sb, in_=wt.rearrange("(ht p) m -> p ht m", p=128))
            uw1 = sb.tile([128, TAGS, F], io)
            nc.sync.dma_start(out=uw1, in_=uw[:, 0:128, :].transpose([1, 0, 2]))
            uw2 = sb.tile([I2, TAGS, F], io)
            nc.sync.dma_start(out=uw2, in_=uw[:, 128:F, :].transpose([1, 0, 2]))
            m_sb = sb.tile([128, NW], io)
            nc.sync.dma_start(out=m_sb, in_=msk.partition_broadcast(128))
            bh1 = sb.tile([128, 1], f32)
            nc.sync.dma_start(out=bh1, in_=bh[0:128].unsqueeze(1))
            bh2 = sb.tile([F2, 1], f32)
            nc.sync.dma_start(out=bh2, in_=bh[128:BSZ].unsqueeze(1))
            bt1 = sb.tile([128, 1], f32)
            nc.sync.dma_start(out=bt1, in_=bt[0:128].unsqueeze(1))
            bt2 = sb.tile([F2, 1], f32)
            nc.sync.dma_start(out=bt2, in_=bt[128:BSZ].unsqueeze(1))

            headT1 = sb.tile([128, NQ], io)
            headT2 = sb.tile([I2, NQ], io)
            tailT1 = sb.tile([128, NW], io)
            tailT2 = sb.tile([I2, NW], io)
            uh1 = sb.tile([128, TAGS, NQ], io)
            uh2 = sb.tile([I2, TAGS, NQ], io)
            s_sb0 = sb.tile([128, TAGS, NQ], f32)
            s_sb1 = sb.tile([128, TAGS, NQ], f32)

            # ---- MLPs: o = gelu(W^T x + b), computed transposed ----
            for w_sb, b1, b2, o1, o2, c0, ncols in (
                (wh_sb, bh1, bh2, headT1, headT2, W, NQ),
                (wt_sb, bt1, bt2, tailT1, tailT2, 0, NW),
            ):
                for fw, f0, o, bias in ((128, 0, o1, b1), (F2, 128, o2, b2)):
                    pm = ps_mlp.tile([fw, ncols], f32, tag="pm")
                    for ht in range(6):
                        nc.tensor.matmul(
                            pm,
                            w_sb[:, ht, f0 : f0 + fw],
                            x_sb[:, ht, c0 : c0 + ncols],
                            start=(ht == 0),
                            stop=(ht == 5),
                        )
                    nc.scalar.activation(out=o[0:fw, :], in_=pm, func=gelu, bias=bias)
                # mask all columns; ones feature row (i == 200) is the mask
                # row itself, DMA'd in (engines can't address partition 72)
                nc.vector.tensor_mul(o1, o1, m_sb[0:128, c0 : c0 + ncols])
                nc.vector.tensor_mul(
                    o2[0:F2, :], o2[0:F2, :], m_sb[0:F2, c0 : c0 + ncols]
                )
                nc.sync.dma_start(
                    out=o2[F2 : F2 + 1, :], in_=msk[:, c0 : c0 + ncols]
                )

            # ---- step A: UhT_t[j, x] = sum_i UW[t,i,j] headT[i,x] ----
            for t in range(TAGS):
                for jw, j0, uh in ((128, 0, uh1), (I2, 128, uh2)):
                    pa = ps_a.tile([jw, NQ], f32, tag="pa")
                    for it, (u_sb, h_sb) in enumerate(
                        ((uw1, headT1), (uw2, headT2))
                    ):
                        nc.tensor.matmul(
                            pa,
                            u_sb[:, t, j0 : j0 + jw],
                            h_sb,
                            start=(it == 0),
                            stop=(it == 1),
                        )
                    nc.any.tensor_copy(uh[:, t, :], pa)

            # ---- step B: S_t[x, m] = sum_j UhT_t[j, x] tailT[j, m] ----
            for qc in range(2):
                s_sb = (s_sb0, s_sb1)[qc]
                for t in range(TAGS):
                    pS = ps_s.tile([128, NQ], f32, tag="ps")
                    for jt, (uh, tl) in enumerate(((uh1, tailT1), (uh2, tailT2))):
                        nc.tensor.matmul(
                            pS,
                            uh[:, t, qc * 128 : qc * 128 + 128],
                            tl[:, qc * 128 : qc * 128 + NQ],
                            start=(jt == 0),
                            stop=(jt == 1),
                        )
                    nc.any.tensor_copy(s_sb[:, t, :], pS)
                nc.sync.dma_start(
                    out=sout[:, qc * 128 : (qc + 1) * 128, :].transpose([1, 0, 2]),
                    in_=s_sb,
                )

    nc.compile()
    return nc


def _np_io_dt():
    if io_dt_name() == "bf16":
        import ml_dtypes

        return ml_dtypes.bfloat16
    return np.float32


def _get_nc():
    key = "nc-" + io_dt_name()
    if key not in _cache:
        _cache[key] = _build_nc()
    return _cache[key]


def _install_ntff_hook():
    """Profiling-only (BASSK_TRACE=1): provide antenv.axon_hooks if the
    image lacks it, wired to the libaxon NTFF capture via ctypes."""
    import sys
    import types

    try:
        from antenv.axon_hooks import get_axon_ntff_profile_hook  # noqa: F401

        return
    except ImportError:
        pass
    from trn_agent_boot.trn_boot import _ntff_profile_via_ctypes

    hook = _ntff_profile_via_ctypes("/opt/axon/libaxon_pjrt.so")
    mod = types.ModuleType("antenv.axon_hooks")
    mod._hook = hook
    mod.get_axon_ntff_profile_hook = lambda: mod._hook
    mod.set_axon_ntff_profile_hook = lambda h: setattr(mod, "_hook", h)
    sys.modules["antenv.axon_hooks"] = mod


def _host_prep(state, lengths, Wh, bh, Wt, bt, U, Wcat, Wd):
    """Fold U/Wcat/Wd into UW[9,201,201] and build per-core inputs."""
    Whp = Wcat[:, :F]  # [K, 201]
    Wtp = Wcat[:, F:]  # [K, 201]
    U2 = U.astype(np.float64).copy()
    U2[:, F - 1, :] += Wtp  # head ones-row picks up the tail term
    U2[:, :, F - 1] += Whp  # tail ones-col picks up the head term
    UW = np.einsum("kt,kij->tij", Wd.astype(np.float64), U2).astype(np.float32)
    UW = np.ascontiguousarray(UW)

    in_maps = []
    for b in range(B):
        for qi in range(N // NQ):
            q0 = qi * NQ
            lo = q0 - W
            xw = np.zeros((NW, HID), np.float32)
            s, e = max(lo, 0), min(q0 + NQ + W, N)
            xw[s - lo : e - lo] = state[b, s:e]
            pos = lo + np.arange(NW)
            mrow = ((pos >= 0) & (pos < N) & (pos < lengths[b])).astype(np.float32)
            iodt = _np_io_dt()
            in_maps.append(
                {
                    "xT": np.ascontiguousarray(xw.T).astype(iodt),
                    "wh": Wh.astype(iodt),
                    "wt": Wt.astype(iodt),
                    "bh": bh,
                    "bt": bt,
                    "uw": UW.astype(iodt),
                    "msk": np.ascontiguousarray(mrow[None, :]).astype(iodt),
                }
            )
    return in_maps


def _assemble(outs, bd):
    """outs: NCORES arrays [TAGS, NQ, NQ] -> scores [B, N, R, TAGS]."""
    scores = np.empty((B, N, R, TAGS), np.float32)
    xg = np.arange(NQ)
    mi = (xg % 128)[:, None] + np.arange(R)[None, :]  # [NQ, R] col gather idx
    for c, S in enumerate(outs):
        b, qi = divmod(c, N // NQ)
        g = np.take_along_axis(S, mi[None, :, :], axis=2)  # [TAGS, NQ, R]
        scores[b, qi * NQ : (qi + 1) * NQ] = g.transpose(1, 2, 0)
    scores += bd.astype(np.float32)[None, None, None, :]
    return np.where(np.isfinite(scores), scores, 0.0).astype(np.float32)


def kernel(**inputs):
    state = np.asarray(inputs["state"], np.float32)
    lengths = np.asarray(inputs["lengths"]).astype(np.int64)
    Wh = np.ascontiguousarray(np.asarray(inputs["Wh"], np.float32))
    bh = np.asarray(inputs["bh"], np.float32)
    Wt = np.ascontiguousarray(np.asarray(inputs["Wt"], np.float32))
    bt = np.asarray(inputs["bt"], np.float32)
    U = np.asarray(inputs["U"], np.float32)
    Wcat = np.asarray(inputs["Wcat"], np.float32)
    Wd = np.asarray(inputs["Wd"], np.float32)
    bd = np.asarray(inputs["bd"], np.float32)

    in_maps = _host_prep(state, lengths, Wh, bh, Wt, bt, U, Wcat, Wd)
    nc = _get_nc()

    if os.environ.get("BASSK_SIM"):
        from concourse.bass_interp import CoreSim

        outs = []
        for im in in_maps:
            sim = CoreSim(nc, trace=False)
            for k, v in im.items():
                sim.tensor(k)[:] = v
            sim.simulate()
            outs.append(sim.tensor("sout").copy())
    else:
        trace = bool(os.environ.get("BASSK_TRACE"))
        if trace:
            _install_ntff_hook()
        from concourse.bass_utils import run_bass_kernel_spmd

        res = run_bass_kernel_spmd(
            nc,
            in_maps,
            core_ids=list(range(NCORES)),
            trace=trace,
        )
        _cache["last_result"] = res
        outs = [r["sout"] for r in res.results]

    return _assemble(outs, bd)


# revision 19
# speedup vs baseline: 1.7773x; 1.3568x over previous
"""Trainium2 Bass kernel for nn_CNNNer (sparse band biaffine NER scorer).

Math collapse used here (everything after the GELU stage is linear):
  head = gelu(state@Wh+bh) ++ [1]          (features i = 0..200, i=200 is the 1)
  tail = gelu(state@Wt+bt) ++ [1]
  band[n,r,k] = head[n]^T U''_k tail[m],  m = n+r-64
      with U''_k = U_k + e_200 Wtp[k,:] + Whp[k,:]^T e_200^T
      (folds the h2/t2 additive terms of scores2 through the ones feature)
  scores'[n,r,t] = sum_k Wd[k,t] band_masked[n,r,k]
      masking zeroes whole head/tail feature columns (query/key validity),
      which commutes with the k-contraction, so
  scores'[n,r,t] = head_masked[n]^T UW_t tail_masked[m],
      UW_t = sum_k Wd[k,t] U''_k            (precomputed on host, [9,201,201])
  scores = scores' + bd  (host), masked-out entries = bd exactly.

Device work per core (8 cores; core = (batch b, query quarter) of 256 queries):
  1. headT/tailT = gelu MLPs computed transposed ([feature, position]).
  2. step A: UhT_t[j, x] = sum_i UW[t,i,j] headT[i,x]        (9 tags)
  3. step B: S_t[x, m]  = sum_j UhT_t[j, x] tailT[j, m]      (full 128x256
     score windows per query-chunk; band diag extracted on host)
"""

import os

import numpy as np

B, N, HID = 2, 1024, 768
BSZ = 200
W = 64
TAGS = 9
F = BSZ + 1  # 201 features incl the ones column
NQ = 256  # queries per core
NW = NQ + 2 * W  # 384 window positions per core
R = 2 * W + 1  # 129 band offsets
NCORES = 8
I2 = F - 128  # 73: second feature tile rows (i = 128..200)
F2 = BSZ - 128  # 72: second MLP output tile cols

_cache: dict = {}


def io_dt_name():
    return os.environ.get("BASSK_IO_DT", "f32r")


def _build_nc():
    import concourse.bass as bass
    import concourse.mybir as mybir
    import concourse.tile as tile
    from concourse import bacc

    dt = mybir.dt
    f32 = dt.float32
    io = {"f32": f32, "f32r": dt.float32r, "bf16": dt.bfloat16}[io_dt_name()]

    nc = bacc.Bacc(
        "TRN2", target_bir_lowering=False, debug=False, enable_asserts=False
    )
    xT = nc.dram_tensor("xT", [HID, NW], io, kind="ExternalInput").ap()
    wh = nc.dram_tensor("wh", [HID, BSZ], io, kind="ExternalInput").ap()
    wt = nc.dram_tensor("wt", [HID, BSZ], io, kind="ExternalInput").ap()
    bh = nc.dram_tensor("bh", [BSZ], f32, kind="ExternalInput").ap()
    bt = nc.dram_tensor("bt", [BSZ], f32, kind="ExternalInput").ap()
    uw = nc.dram_tensor("uw", [TAGS, F, F], io, kind="ExternalInput").ap()
    msk = nc.dram_tensor("msk", [1, NW], io, kind="ExternalInput").ap()
    sout = nc.dram_tensor("sout", [TAGS, NQ, NQ], f32, kind="ExternalOutput").ap()

    gelu = {
        "gelu": mybir.ActivationFunctionType.Gelu,
        "identity": mybir.ActivationFunctionType.Identity,
    }[os.environ.get("BASSK_ACT", "gelu")]

    with tile.TileContext(nc) as tc:
        with (
            tc.tile_pool(name="sb", bufs=1) as sb,
            tc.tile_pool(name="ps_mlp", bufs=2, space="PSUM") as ps_mlp,
            tc.tile_pool(name="ps_a", bufs=2, space="PSUM") as ps_a,
            tc.tile_pool(name="ps_s", bufs=4, space="PSUM") as ps_s,
        ):
            # ---- loads (spread across DGE queues; x/weights split so the
            # MLP matmuls can start on the first chunks) ----
            xTr = xT.rearrange("(ht p) c -> p ht c", p=128)
            x_sb = sb.tile([128, 6, NW], io)
            nc.sync.dma_start(out=x_sb[:, 0:2, :], in_=xTr[:, 0:2, :])
            nc.scalar.dma_start(out=x_sb[:, 2:4, :], in_=xTr[:, 2:4, :])
            nc.gpsimd.dma_start(out=x_sb[:, 4:6, :], in_=xTr[:, 4:6, :])
            wh_sb = sb.tile([128, 6, BSZ], io)
            nc.sync.dma_start(out=wh_sb, in_=wh.rearrange("(ht p) m -> p ht m", p=128))
            wt_sb = sb.tile([128, 6, BSZ], io)
            nc.scalar.dma_start(
                out=wt_sb, in_=wt.rearrange("(ht p) m -> p ht m", p=128)
            )
            uw1 = sb.tile([128, TAGS, F], io)
            nc.scalar.dma_start(out=uw1, in_=uw[:, 0:128, :].transpose([1, 0, 2]))
            uw2 = sb.tile([I2, TAGS, F], io)
            nc.sync.dma_start(out=uw2, in_=uw[:, 128:F, :].transpose([1, 0, 2]))
            m_sb = sb.tile([128, NW], io)
            nc.gpsimd.dma_start(out=m_sb, in_=msk.partition_broadcast(128))
            bh1 = sb.tile([128, 1], f32)
            nc.gpsimd.dma_start(out=bh1, in_=bh[0:128].unsqueeze(1))
            bh2 = sb.tile([F2, 1], f32)
            nc.gpsimd.dma_start(out=bh2, in_=bh[128:BSZ].unsqueeze(1))
            bt1 = sb.tile([128, 1], f32)
            nc.gpsimd.dma_start(out=bt1, in_=bt[0:128].unsqueeze(1))
            bt2 = sb.tile([F2, 1], f32)
            nc.gpsimd.dma_start(out=bt2, in_=bt[128:BSZ].unsqueeze(1))

            headT1 = sb.tile([128, NQ], io)
            headT2 = sb.tile([I2, NQ], io)
            tailT1 = sb.tile([128, NW], io)
            tailT2 = sb.tile([I2, NW], io)
            uh1 = sb.tile([128, TAGS, NQ], io)
            uh2 = sb.tile([I2, TAGS, NQ], io)
            s_sb0 = sb.tile([128, TAGS, NQ], f32)
            s_sb1 = sb.tile([128, TAGS, NQ], f32)

            # ---- MLPs: o = gelu(W^T x + b), computed transposed ----
            for w_sb, b1, b2, o1, o2, c0, ncols in (
                (wh_sb, bh1, bh2, headT1, headT2, W, NQ),
                (wt_sb, bt1, bt2, tailT1, tailT2, 0, NW),
            ):
                for fw, f0, o, bias in ((128, 0, o1, b1), (F2, 128, o2, b2)):
                    pm = ps_mlp.tile([fw, ncols], f32, tag="pm")
                    for ht in range(6):
                        nc.tensor.matmul(
                            pm,
                            w_sb[:, ht, f0 : f0 + fw],
                            x_sb[:, ht, c0 : c0 + ncols],
                            start=(ht == 0),
                            stop=(ht == 5),
                        )
                    nc.scalar.activation(out=o[0:fw, :], in_=pm, func=gelu, bias=bias)
                # mask all columns; ones feature row (i == 200) is the mask
                # row itself, DMA'd in (engines can't address partition 72)
                nc.vector.tensor_mul(o1, o1, m_sb[0:128, c0 : c0 + ncols])
                nc.vector.tensor_mul(
                    o2[0:F2, :], o2[0:F2, :], m_sb[0:F2, c0 : c0 + ncols]
                )
                nc.gpsimd.dma_start(
                    out=o2[F2 : F2 + 1, :], in_=msk[:, c0 : c0 + ncols]
                )

            # ---- step A: UhT_t[j, x] = sum_i UW[t,i,j] headT[i,x] ----
            for t in range(TAGS):
                for jw, j0, uh in ((128, 0, uh1), (I2, 128, uh2)):
                    pa = ps_a.tile([jw, NQ], f32, tag="pa")
                    for it, (u_sb, h_sb) in enumerate(
                        ((uw1, headT1), (uw2, headT2))
                    ):
                        nc.tensor.matmul(
                            pa,
                            u_sb[:, t, j0 : j0 + jw],
                            h_sb,
                            start=(it == 0),
                            stop=(it == 1),
                        )
                    nc.any.tensor_copy(uh[:, t, :], pa)

            # ---- step B: S_t[x, m] = sum_j UhT_t[j, x] tailT[j, m] ----
            for qc in range(2):
                s_sb = (s_sb0, s_sb1)[qc]
                for t in range(TAGS):
                    pS = ps_s.tile([128, NQ], f32, tag="ps")
                    for jt, (uh, tl) in enumerate(((uh1, tailT1), (uh2, tailT2))):
                        nc.tensor.matmul(
                            pS,
                            uh[:, t, qc * 128 : qc * 128 + 128],
                            tl[:, qc * 128 : qc * 128 + NQ],
                            start=(jt == 0),
                            stop=(jt == 1),
                        )
                    nc.any.tensor_copy(s_sb[:, t, :], pS)
                eng = (nc.sync, nc.scalar)[qc]
                eng.dma_start(
                    out=sout[:, qc * 128 : (qc + 1) * 128, :].transpose([1, 0, 2]),
                    in_=s_sb,
                )

    nc.compile()
    return nc


def _np_io_dt():
    if io_dt_name() == "bf16":
        import ml_dtypes

        return ml_dtypes.bfloat16
    return np.float32


def _get_nc():
    key = "nc-" + io_dt_name()
    if key not in _cache:
        _cache[key] = _build_nc()
    return _cache[key]


def _install_ntff_hook():
    """Profiling-only (BASSK_TRACE=1): provide antenv.axon_hooks if the
    image lacks it, wired to the libaxon NTFF capture via ctypes."""
    import sys
    import types

    try:
        from antenv.axon_hooks import get_axon_ntff_profile_hook  # noqa: F401

        return
    except ImportError:
        pass
    from trn_agent_boot.trn_boot import _ntff_profile_via_ctypes

    hook = _ntff_profile_via_ctypes("/opt/axon/libaxon_pjrt.so")
    mod = types.ModuleType("antenv.axon_hooks")
    mod._hook = hook
    mod.get_axon_ntff_profile_hook = lambda: mod._hook
    mod.set_axon_ntff_profile_hook = lambda h: setattr(mod, "_hook", h)
    sys.modules["antenv.axon_hooks"] = mod


def _host_prep(state, lengths, Wh, bh, Wt, bt, U, Wcat, Wd):
    """Fold U/Wcat/Wd into UW[9,201,201] and build per-core inputs."""
    Whp = Wcat[:, :F]  # [K, 201]
    Wtp = Wcat[:, F:]  # [K, 201]
    U2 = U.astype(np.float64).copy()
    U2[:, F - 1, :] += Wtp  # head ones-row picks up the tail term
    U2[:, :, F - 1] += Whp  # tail ones-col picks up the head term
    UW = np.einsum("kt,kij->tij", Wd.astype(np.float64), U2).astype(np.float32)
    UW = np.ascontiguousarray(UW)

    in_maps = []
    for b in range(B):
        for qi in range(N // NQ):
            q0 = qi * NQ
            lo = q0 - W
            xw = np.zeros((NW, HID), np.float32)
            s, e = max(lo, 0), min(q0 + NQ + W, N)
            xw[s - lo : e - lo] = state[b, s:e]
            pos = lo + np.arange(NW)
            mrow = ((pos >= 0) & (pos < N) & (pos < lengths[b])).astype(np.float32)
            iodt = _np_io_dt()
            in_maps.append(
                {
                    "xT": np.ascontiguousarray(xw.T).astype(iodt),
                    "wh": Wh.astype(iodt),
                    "wt": Wt.astype(iodt),
                    "bh": bh,
                    "bt": bt,
                    "uw": UW.astype(iodt),
                    "msk": np.ascontiguousarray(mrow[None, :]).astype(iodt),
                }
            )
    return in_maps


def _assemble(outs, bd):
    """outs: NCORES arrays [TAGS, NQ, NQ] -> scores [B, N, R, TAGS]."""
    scores = np.empty((B, N, R, TAGS), np.float32)
    mi = (np.arange(NQ) % 128)[:, None] + np.arange(R)[None, :]
    for c, S in enumerate(outs):
        b, qi = divmod(c, N // NQ)
        g = np.take_along_axis(S, mi[None, :, :], axis=2)
        scores[b, qi * NQ : (qi + 1) * NQ] = g.transpose(1, 2, 0)
    scores += bd.astype(np.float32)[None, None, None, :]
    return np.where(np.isfinite(scores), scores, 0.0).astype(np.float32)


def kernel(**inputs):
    state = np.asarray(inputs["state"], np.float32)
    lengths = np.asarray(inputs["lengths"]).astype(np.int64)
    Wh = np.ascontiguousarray(np.asarray(inputs["Wh"], np.float32))
    bh = np.asarray(inputs["bh"], np.float32)
    Wt = np.ascontiguousarray(np.asarray(inputs["Wt"], np.float32))
    bt = np.asarray(inputs["bt"], np.float32)
    U = np.asarray(inputs["U"], np.float32)
    Wcat = np.asarray(inputs["Wcat"], np.float32)
    Wd = np.asarray(inputs["Wd"], np.float32)
    bd = np.asarray(inputs["bd"], np.float32)

    in_maps = _host_prep(state, lengths, Wh, bh, Wt, bt, U, Wcat, Wd)
    nc = _get_nc()

    if os.environ.get("BASSK_SIM"):
        from concourse.bass_interp import CoreSim

        outs = []
        for im in in_maps:
            sim = CoreSim(nc, trace=False)
            for k, v in im.items():
                sim.tensor(k)[:] = v
            sim.simulate()
            outs.append(sim.tensor("sout").copy())
    else:
        trace = bool(os.environ.get("BASSK_TRACE"))
        if trace:
            _install_ntff_hook()
        from concourse.bass_utils import run_bass_kernel_spmd

        res = run_bass_kernel_spmd(
            nc,
            in_maps,
            core_ids=list(range(NCORES)),
            trace=trace,
        )
        _cache["last_result"] = res
        outs = [r["sout"] for r in res.results]

    return _assemble(outs, bd)
